# revision 39
# baseline (speedup 1.0000x reference)
"""CRF negative-log-likelihood kernel for Trainium2, SPMD over 8 NeuronCores.

v5 strategy
-----------
Data-parallel over batch: core c handles sequences b in [c*8, (c+1)*8).

Per core (B=8 local sequences, T=512, K=50 tags, D=1024):

1. Hidden states arrive PRE-TRANSPOSED and BLOCKED from the host as
   hidtb[block, dchunk, 128d, seq, 64t] bf16 -- one contiguous 1MB DMA
   per 64-column block (8 DMAs, 2 queues). No device transposes.

2. Emissions GEMM (bf16): per 64-col block (all 8 seqs at once):
   8 accumulating d-chunk matmuls with 512-wide moving [8 seq x 64 t]
   into one PSUM bank; column-DOUBLED stationary W2 puts emisT on both
   partition row-blocks. Act-exp with bias (b - c) produces E, where
   c = ln sum_k exp(b_k + ||W_k||^2/2) keeps the recurrence
   growth-neutral with NO per-column normalisation (exact math:
   the -c factors cancel between ln Z and the gold emission term).

3. E storage: rows 0:64 hold E_t at column t, rows 64:128 hold
   E_{511-t} at column t (time reversed), so one merged fwd/bwd chain
   step reads a single column.

4. TWO merged chains run concurrently (interleaved on PE+DVE):
     chain 1: cols 1..127   (fwd t=1..127   / bwd t=510..384)
     chain 2: cols 121..255 (fwd t=121..255 / bwd t=390..256),
              warm-started at col 120 with state := E2[:, :, 120].
   The CRF step matrix mixes with contraction ~0.03/step, so after
   chain 2's 7 warm-up steps its state direction is exact to ~1e-11;
   the unknown warm-start SCALE is removed exactly by the ratio
     (1.alpha_127)(1.gamma_384) / (1.alpha^_127)(1.gamma^_384)
   using chain 1's final state and a snapshot of chain 2 at col 127.
   log Z = ln(alpha^_255 . A gamma^_256) + ln-ratio  (+ T*c, which
   cancels against the gold accumulator).  ~136 rounds instead of 255.

5. Gold path: device computes only sum_t emis[tag_t, t] (one-hot via
   rank-1 tag broadcast + iota-compare in prescan; GpSimd multiply +
   Scalar accumulate pumped into scan gaps). Transition/start/end gold
   score is computed on the host from tag_ids alone.
"""

import numpy as np

B_FULL = 64
B_LOC = 8
T = 512
K = 50
D = 1024
N_CORES = 8
H2 = 64   # partition base of the bwd/second row block
NDC = D // 128  # 8 d-chunks
NB = 8    # t-blocks
BT = T // NB    # 64 cols per block
WARM = 120      # chain 2 warm-start column
C1END = 127     # chain 1 final column (also the handoff column)
C2END = 255     # chain 2 final column

_COMPILED = {}
LAST_RESULT = None


def _build(dbg=False):
    import os

    import concourse.bass as bass
    import concourse.tile as tile
    from concourse import bacc, mybir

    pump_mode = int(os.environ.get("V2_PUMP", "2"))  # 0=no interleave

    f32 = mybir.dt.float32
    bf16 = mybir.dt.bfloat16
    fp8 = mybir.dt.float8e4

    nc = bacc.Bacc(
        "TRN2",
        target_bir_lowering=False,
        debug=False,
        num_devices=N_CORES,
    )

    def flip_last(ap):
        """Reverse the innermost free dim of an AP (negative stride)."""
        st, n = ap.ap[-1]
        return bass.AP(ap.tensor, ap.offset + (n - 1) * st,
                       ap.ap[:-1] + [[-st, n]])

    hidtb = nc.dram_tensor("hidtb", [NB, 128, NDC, B_LOC, BT], fp8,
                           kind="ExternalInput")
    wq2 = nc.dram_tensor("wq2", [8, 128, 128], bf16, kind="ExternalInput")
    s2 = nc.dram_tensor("s2", [128, 128], bf16, kind="ExternalInput")
    ident = nc.dram_tensor("ident", [128, 128], f32, kind="ExternalInput")
    # cols: 0=initcol(exp st | exp en) 3=bcol(b-c) 4=iota 5=ones(0:K)
    cols = nc.dram_tensor("cols", [128, 8], f32, kind="ExternalInput")
    # ones2b: col0 = ones on rows 0:K, col1 = ones on rows H2:H2+K (bf16)
    ones2b = nc.dram_tensor("ones2b", [128, 2], bf16, kind="ExternalInput")
    onesrow_b = nc.dram_tensor("onesrow_b", [1, 128], bf16, kind="ExternalInput")
    tagrow = nc.dram_tensor("tagrow", [1, B_LOC * T], bf16, kind="ExternalInput")
    out_d = nc.dram_tensor("out", [1, B_LOC], f32, kind="ExternalOutput")
    if dbg:
        dbg_st = nc.dram_tensor("dbg_st", [128, 4 * B_LOC], f32,
                                kind="ExternalOutput")

    AF = mybir.ActivationFunctionType
    ALU = mybir.AluOpType

    with tile.TileContext(nc) as tc:
        with (
            tc.tile_pool(name="consts", bufs=1) as consts,
            tc.tile_pool(name="persist", bufs=1) as persist,
            tc.tile_pool(name="al1", bufs=4) as al1_pool,
            tc.tile_pool(name="al2", bufs=4) as al2_pool,
            tc.tile_pool(name="srow", bufs=6) as srow_pool,
            tc.tile_pool(name="ge_ps", bufs=2, space=bass.MemorySpace.PSUM) as ge_ps,
            tc.tile_pool(name="s1_ps", bufs=2, space=bass.MemorySpace.PSUM) as s1_ps,
            tc.tile_pool(name="s2_ps", bufs=2, space=bass.MemorySpace.PSUM) as s2_ps,
        ):
            # ---- constants (warm-up deps first: tag/onesrow/cols/w2/s2) ----
            tag_sb = consts.tile([1, B_LOC * T], bf16)
            nc.sync.dma_start(tag_sb[:], tagrow[:])
            onesrow_b_sb = consts.tile([1, 128], bf16)
            nc.sync.dma_start(onesrow_b_sb[:], onesrow_b[:])
            cols_sb = consts.tile([128, 8], f32)
            nc.sync.dma_start(cols_sb[:], cols[:])
            w2_sb = consts.tile([128, 8, 128], bf16)
            nc.sync.dma_start(w2_sb[:], wq2[:].rearrange("c p k -> p c k"))
            s2_sb = consts.tile([128, 128], bf16)
            nc.sync.dma_start(s2_sb[:], s2[:])
            id_sb = consts.tile([128, 128], f32)
            nc.sync.dma_start(id_sb[:], ident[:])
            ones2_sb = consts.tile([128, 2], bf16)
            nc.sync.dma_start(ones2_sb[:], ones2b[:])

            initcol = cols_sb[:, 0:1]
            bcol = cols_sb[:, 3:4]
            iota = cols_sb[:, 4:5]
            onescol_f = cols_sb[:, 5:6]

            # ---- persistent tiles ----
            # one tile per 64-col block so each GEMM depends only on its DMA
            hts = [
                persist.tile([128, NDC, B_LOC, BT], fp8, name=f"hts{k}")
                for k in range(NB)
            ]
            E2 = persist.tile([128, B_LOC, T], bf16)    # E (rows 64+ reversed)
            emis = persist.tile([128, B_LOC, T], bf16)  # raw emisT+(b-c), rows 0:K
            OH = persist.tile([128, B_LOC, T], bf16)    # one-hot (rows 0:K)
            g1 = persist.tile([128, B_LOC], f32)        # gold emission term
            scr2 = persist.tile([128, T], bf16)         # accum scratch dst
            snap2 = persist.tile([128, B_LOC], bf16)    # chain2 state at col 127
            betas = persist.tile([128, B_LOC], f32)
            wdot = persist.tile([128, B_LOC], f32)

            # ---- stage all hidden data: one DMA per 64-col block ----
            # ONE queue => per-queue in-order completion, so the first
            # blocks land early and the GEMM can stream behind the DMA.
            for k in (0, 7, 1, 6, 2, 5, 3, 4):
                nc.sync.dma_start(hts[k][:], hidtb[k])

            # ---- emissions GEMM: one unit per 64-col block (all seqs) ----
            def unit_blk(k):
                kc = slice(k * BT, (k + 1) * BT)
                rkc = slice((NB - 1 - k) * BT, (NB - k) * BT)
                pe_ = ge_ps.tile([128, B_LOC, BT], f32, tag="ge")
                for dc in range(8):
                    nc.tensor.matmul(
                        pe_[:],
                        w2_sb[:, dc, :],
                        hts[k][:, dc, :, :],
                        start=(dc == 0),
                        stop=(dc == 7),
                    )
                    if dc in (2, 5):
                        yield
                yield
                nc.scalar.activation(
                    E2[0:H2, :, kc], pe_[0:H2, :, :], AF.Exp, bias=bcol[0:H2]
                )
                nc.scalar.activation(
                    E2[H2:128, :, rkc], flip_last(pe_[H2:128, :, :]),
                    AF.Exp, bias=bcol[H2:128],
                )
                yield
                # raw (bias-free) emissions for the gold path; the host adds
                # sum_t b[tag_t] and the T*c shift itself.
                nc.vector.tensor_copy(emis[0:K, :, kc], pe_[0:K, :, :])
                yield

            def unit_goldoh(k):
                # one-hot build for 64-col block k, all seqs
                kc = slice(k * BT, (k + 1) * BT)
                tagap = tag_sb[:].rearrange("p (c t) -> p c t", c=B_LOC)[:, :, kc]
                tb = ge_ps.tile([128, B_LOC, BT], f32, tag="ge")
                nc.tensor.matmul(
                    tb[0:K, :, :], onesrow_b_sb[:, 0:K], tagap,
                    start=True, stop=True,
                )
                yield
                nc.vector.tensor_scalar(
                    OH[0:K, :, kc], tb[0:K, :, :], iota[0:K], None,
                    ALU.is_equal,
                )
                yield

            def unit_goldmul(c):
                nc.gpsimd.tensor_mul(
                    OH[0:K, c, :], emis[0:K, c, :], OH[0:K, c, :]
                )
                yield
                nc.scalar.activation(
                    scr2[0:K, 0:T], OH[0:K, c, :],
                    AF.Identity, accum_out=g1[0:K, c : c + 1],
                )
                yield

            # ---- PE warm-up while the first DMA is in flight ----
            # gold one-hots first (no hidtb dependency), then junk const
            # matmuls: keeps the PE p-state ramping so the real GEMM runs
            # at full clock as soon as block 0 lands.
            for k in range(NB):
                for _ in unit_goldoh(k):
                    pass
            jk_ps = ge_ps.tile([128, 512], f32, tag="jk", name="jk")
            for _ in range(2):
                nc.tensor.matmul(
                    jk_ps[:], s2_sb[:],
                    w2_sb[:].rearrange("p c k -> p (c k)")[:, 0:512],
                    start=True, stop=True,
                )

            # ---- pre-scan: all GEMM blocks ----
            for k in (0, 7, 1, 6, 2, 5, 3, 4):
                for _ in unit_blk(k):
                    pass

            # ---- chain inits ----
            al1 = al1_pool.tile([128, B_LOC], bf16, tag="a1")
            nc.vector.tensor_scalar_mul(al1[:], E2[:, :, 0], initcol)
            al2 = al2_pool.tile([128, B_LOC], bf16, tag="a2")
            nc.vector.tensor_copy(al2[:], E2[:, :, WARM])

            # gold multiplies/accums pumped into scan gaps
            work = [unit_goldmul(c) for c in range(B_LOC)]

            def pump(n):
                for _ in range(n):
                    while work:
                        try:
                            next(work[0])
                            break
                        except StopIteration:
                            work.pop(0)

            if pump_mode == 0:
                pump(len(work) * 16)

            # ---- two merged fwd/bwd chains, interleaved ----
            for j in range(1, C2END - WARM + 1):
                if j <= C1END:  # chain 1: col j
                    ps1 = s1_ps.tile([128, B_LOC], f32, tag="s1")
                    nc.tensor.matmul(ps1[:], s2_sb[:], al1[:],
                                     start=True, stop=True)
                    al1_new = al1_pool.tile([128, B_LOC], bf16, tag="a1")
                    nc.vector.tensor_mul(al1_new[:], ps1[:], E2[:, :, j])
                    al1 = al1_new
                col2 = WARM + j  # chain 2
                ps2 = s2_ps.tile([128, B_LOC], f32, tag="s2")
                nc.tensor.matmul(ps2[:], s2_sb[:], al2[:],
                                 start=True, stop=True)
                al2_new = al2_pool.tile([128, B_LOC], bf16, tag="a2")
                nc.vector.tensor_mul(al2_new[:], ps2[:], E2[:, :, col2])
                al2 = al2_new
                if col2 == C1END:  # snapshot chain 2 at the handoff column
                    nc.vector.tensor_copy(snap2[:], al2[:])
                if pump_mode:
                    pump(pump_mode)

            pump(len(work) * 16)  # drain remaining background work

            # ---- finisher ----
            # z = alpha^_255 . (A gamma^_256) on chain 2's final state
            ps_f = s1_ps.tile([128, B_LOC], f32, tag="s1")
            nc.tensor.matmul(ps_f[:], s2_sb[:], al2[:], start=True, stop=True)
            nc.vector.tensor_copy(betas[H2 : H2 + K, :], ps_f[H2 : H2 + K, :])
            psz = s1_ps.tile([128, B_LOC], f32, tag="s1")
            nc.tensor.matmul(
                psz[0:K, :], id_sb[H2 : H2 + K, H2 : H2 + K],
                betas[H2 : H2 + K, :], start=True, stop=True,
            )
            nc.vector.tensor_mul(wdot[0:K, :], psz[0:K, :], al2[0:K, :])
            zz = s1_ps.tile([128, B_LOC], f32, tag="s1")
            nc.tensor.matmul(zz[0:1, :], onescol_f[0:K], wdot[0:K, :],
                             start=True, stop=True)
            lnz = srow_pool.tile([1, B_LOC], f32, tag="srow")
            nc.scalar.activation(lnz[:], zz[0:1, :], AF.Ln)

            # scale-ratio correction: + ln(1.a_127)(1.g_384) - ln(^ version)
            def lnsum2(state_bf16):
                ps_r = s2_ps.tile([128, B_LOC], f32, tag="s2")
                nc.tensor.matmul(ps_r[0:2, :], ones2_sb[:], state_bf16,
                                 start=True, stop=True)
                lt = srow_pool.tile([2, B_LOC], f32, tag="srow")
                nc.scalar.activation(lt[:], ps_r[0:2, :], AF.Ln)
                ps_s = s2_ps.tile([128, B_LOC], f32, tag="s2")
                nc.tensor.matmul(ps_s[0:1, :], onescol_f[0:2], lt[:],
                                 start=True, stop=True)
                row = srow_pool.tile([1, B_LOC], f32, tag="srow")
                nc.vector.tensor_copy(row[:], ps_s[0:1, :])
                return row

            lnp1 = lnsum2(al1[:])     # chain 1 final (true state at handoff)
            lnp2 = lnsum2(snap2[:])   # chain 2 snapshot (hatted state)

            if dbg:
                dstate = persist.tile([128, 4 * B_LOC], f32)
                nc.vector.tensor_copy(dstate[:, 0:B_LOC], al1[:])
                nc.vector.tensor_copy(dstate[:, B_LOC : 2 * B_LOC], snap2[:])
                nc.vector.tensor_copy(dstate[:, 2 * B_LOC : 3 * B_LOC], al2[:])
                nc.sync.dma_start(dbg_st[:], dstate[:])

            # gold total (emission term only; host adds transition score)
            gzz = s1_ps.tile([128, B_LOC], f32, tag="s1")
            nc.tensor.matmul(gzz[0:1, :], onescol_f[0:K], g1[0:K, :],
                             start=True, stop=True)

            outrow = srow_pool.tile([1, B_LOC], f32, tag="srow")
            nc.vector.tensor_add(outrow[:], lnz[:], lnp1[:])
            nc.vector.tensor_sub(outrow[:], outrow[:], lnp2[:])
            nc.vector.tensor_sub(outrow[:], outrow[:], gzz[0:1, :])
            nc.sync.dma_start(out_d[:], outrow[:])

    nc.compile()
    return nc


def _get_compiled(dbg=False):
    key = ("dbg" if dbg else "nc")
    if key not in _COMPILED:
        _COMPILED[key] = _build(dbg)
    return _COMPILED[key]


def _host_inputs(W, b, transitions, start_trans, end_trans):
    import ml_dtypes

    bf16 = ml_dtypes.bfloat16
    expA = np.exp(transitions).astype(np.float32)
    s2 = np.zeros((128, 128), np.float32)
    s2[0:K, 0:K] = expA
    s2[H2 : H2 + K, H2 : H2 + K] = expA.T

    wq2 = np.zeros((8, 128, 128), np.float32)
    wr = W.reshape(8, 128, K)
    wq2[:, :, 0:K] = wr
    wq2[:, :, H2 : H2 + K] = wr

    # growth-neutralising constant: E[colsum of exp(emis+b)] for h ~ N(0, I)
    c_shift = float(
        np.log(np.sum(np.exp(b.astype(np.float64)
                             + 0.5 * np.sum(W.astype(np.float64) ** 2, axis=0))))
    )

    cols = np.zeros((128, 8), np.float32)
    cols[0:K, 0] = np.exp(start_trans)
    cols[H2 : H2 + K, 0] = np.exp(end_trans)
    cols[0:K, 3] = b - c_shift
    cols[H2 : H2 + K, 3] = b - c_shift
    cols[0:K, 4] = np.arange(K, dtype=np.float32)
    cols[0:K, 5] = 1.0

    ones2 = np.zeros((128, 2), np.float32)
    ones2[0:K, 0] = 1.0
    ones2[H2 : H2 + K, 1] = 1.0

    common = {
        "wq2": np.ascontiguousarray(wq2.astype(bf16)),
        "s2": np.ascontiguousarray(s2.astype(bf16)),
        "ident": np.eye(128, dtype=np.float32),
        "cols": np.ascontiguousarray(cols),
        "ones2b": np.ascontiguousarray(ones2.astype(bf16)),
        "onesrow_b": np.ones((1, 128), bf16),
    }
    return common, c_shift


def kernel(full_hidden, tag_ids, mask, W, b, transitions, start_trans, end_trans,
           dbg=False):
    global LAST_RESULT
    import ml_dtypes
    from concourse.bass_utils import run_bass_kernel_spmd

    bf16 = ml_dtypes.bfloat16
    full_hidden = np.asarray(full_hidden, dtype=np.float32)
    tags = np.asarray(tag_ids)
    W = np.asarray(W, dtype=np.float32)
    b = np.asarray(b, dtype=np.float32)
    transitions = np.asarray(transitions, dtype=np.float32)
    start_trans = np.asarray(start_trans, dtype=np.float32)
    end_trans = np.asarray(end_trans, dtype=np.float32)

    nc = _get_compiled(dbg)
    common, c_shift = _host_inputs(W, b, transitions, start_trans, end_trans)

    # pre-transposed + blocked hidden, block-major and partition-major
    hb = full_hidden.astype(ml_dtypes.float8_e4m3fn)  # [B, T, D]
    hbt = hb.transpose(0, 2, 1).reshape(B_FULL, NDC, 128, NB, BT)
    hbt = hbt.transpose(3, 2, 1, 0, 4)               # [NB, 128, NDC, B, BT]

    in_maps = []
    for c in range(N_CORES):
        sl = slice(c * B_LOC, (c + 1) * B_LOC)
        in_maps.append(
            {
                "hidtb": np.ascontiguousarray(hbt[:, :, :, sl, :]),  # [NB,128,NDC,B,BT]
                "tagrow": np.ascontiguousarray(
                    tags[sl].astype(np.float32).reshape(1, B_LOC * T).astype(bf16)
                ),
                **common,
            }
        )

    # host-side gold transition/start/end score (depends only on tag_ids/mask)
    m = np.asarray(mask).astype(bool)
    tg = tags.astype(np.int64)
    first = tg[:, 0]
    tscore = start_trans[first].astype(np.float64)
    prev = first.copy()
    for t in range(1, T):
        step = transitions[prev, tg[:, t]]
        tscore = np.where(m[:, t], tscore + step, tscore)
        prev = np.where(m[:, t], tg[:, t], prev)
    tscore = tscore + end_trans[prev]
    # device gold accumulates RAW emissions (no bias, no -c shift), while
    # ln z carries -T*c from the E bias; reconcile both here.
    tscore = tscore + b.astype(np.float64)[tg].sum(axis=1) - T * c_shift

    res = run_bass_kernel_spmd(nc, in_maps, core_ids=list(range(N_CORES)))
    LAST_RESULT = res
    out = np.concatenate(
        [np.asarray(res.results[c]["out"]).reshape(B_LOC) for c in range(N_CORES)]
    )
    # -c_shift bias cancels between ln z (-T*c) and the gold accumulator.
    return (out.astype(np.float64) - tscore).astype(np.float32)


# revision 42
# speedup vs baseline: 1.2056x; 1.2056x over previous
"""CRF negative-log-likelihood kernel for Trainium2, SPMD over 8 NeuronCores.

v5 strategy
-----------
Data-parallel over batch: core c handles sequences b in [c*8, (c+1)*8).

Per core (B=8 local sequences, T=512, K=50 tags, D=1024):

1. Hidden states arrive PRE-TRANSPOSED and BLOCKED from the host as
   hidtb[block, dchunk, 128d, seq, 64t] bf16 -- one contiguous 1MB DMA
   per 64-column block (8 DMAs, 2 queues). No device transposes.

2. Emissions GEMM (bf16): per 64-col block (all 8 seqs at once):
   8 accumulating d-chunk matmuls with 512-wide moving [8 seq x 64 t]
   into one PSUM bank; column-DOUBLED stationary W2 puts emisT on both
   partition row-blocks. Act-exp with bias (b - c) produces E, where
   c = ln sum_k exp(b_k + ||W_k||^2/2) keeps the recurrence
   growth-neutral with NO per-column normalisation (exact math:
   the -c factors cancel between ln Z and the gold emission term).

3. E storage: rows 0:64 hold E_t at column t, rows 64:128 hold
   E_{511-t} at column t (time reversed), so one merged fwd/bwd chain
   step reads a single column.

4. TWO merged chains run concurrently (interleaved on PE+DVE):
     chain 1: cols 1..127   (fwd t=1..127   / bwd t=510..384)
     chain 2: cols 121..255 (fwd t=121..255 / bwd t=390..256),
              warm-started at col 120 with state := E2[:, :, 120].
   The CRF step matrix mixes with contraction ~0.03/step, so after
   chain 2's 7 warm-up steps its state direction is exact to ~1e-11;
   the unknown warm-start SCALE is removed exactly by the ratio
     (1.alpha_127)(1.gamma_384) / (1.alpha^_127)(1.gamma^_384)
   using chain 1's final state and a snapshot of chain 2 at col 127.
   log Z = ln(alpha^_255 . A gamma^_256) + ln-ratio  (+ T*c, which
   cancels against the gold accumulator).  ~136 rounds instead of 255.

5. Gold path: device computes only sum_t emis[tag_t, t] (one-hot via
   rank-1 tag broadcast + iota-compare in prescan; GpSimd multiply +
   Scalar accumulate pumped into scan gaps). Transition/start/end gold
   score is computed on the host from tag_ids alone.
"""

import numpy as np

B_FULL = 64
B_LOC = 8
T = 512
K = 50
D = 1024
N_CORES = 8
H2 = 64   # partition base of the bwd/second row block
NDC = D // 128  # 8 d-chunks
NB = 8    # t-blocks
BT = T // NB    # 64 cols per block
NCH = 3         # concurrent merged chains
WARMN = 7       # warm-up steps per handoff

_COMPILED = {}
LAST_RESULT = None


def _build(dbg=False):
    import os

    import concourse.bass as bass
    import concourse.tile as tile
    from concourse import bacc, mybir

    pump_mode = int(os.environ.get("V2_PUMP", "2"))  # 0=no interleave

    f32 = mybir.dt.float32
    bf16 = mybir.dt.bfloat16
    fp8 = mybir.dt.float8e4

    nc = bacc.Bacc(
        "TRN2",
        target_bir_lowering=False,
        debug=False,
        num_devices=N_CORES,
    )

    def flip_last(ap):
        """Reverse the innermost free dim of an AP (negative stride)."""
        st, n = ap.ap[-1]
        return bass.AP(ap.tensor, ap.offset + (n - 1) * st,
                       ap.ap[:-1] + [[-st, n]])

    hidtb = nc.dram_tensor("hidtb", [NB, 128, NDC, B_LOC, BT], fp8,
                           kind="ExternalInput")
    wq2 = nc.dram_tensor("wq2", [8, 128, 128], bf16, kind="ExternalInput")
    s2 = nc.dram_tensor("s2", [128, 128], bf16, kind="ExternalInput")
    ident = nc.dram_tensor("ident", [128, 128], f32, kind="ExternalInput")
    # cols: 0=initcol(exp st | exp en) 3=bcol(b-c) 4=iota 5=ones(0:K)
    cols = nc.dram_tensor("cols", [128, 8], f32, kind="ExternalInput")
    # ones2b: col0 = ones on rows 0:K, col1 = ones on rows H2:H2+K (bf16)
    ones2b = nc.dram_tensor("ones2b", [128, 2], bf16, kind="ExternalInput")
    onesrow_b = nc.dram_tensor("onesrow_b", [1, 128], bf16, kind="ExternalInput")
    tagrow = nc.dram_tensor("tagrow", [1, B_LOC * T], bf16, kind="ExternalInput")
    out_d = nc.dram_tensor("out", [1, B_LOC], f32, kind="ExternalOutput")
    if dbg:
        dbg_st = nc.dram_tensor("dbg_st", [128, 4 * B_LOC], f32,
                                kind="ExternalOutput")

    AF = mybir.ActivationFunctionType
    ALU = mybir.AluOpType

    with tile.TileContext(nc) as tc:
        with (
            tc.tile_pool(name="consts", bufs=1) as consts,
            tc.tile_pool(name="persist", bufs=1) as persist,
            tc.tile_pool(name="al0", bufs=4) as al0_pool,
            tc.tile_pool(name="al1", bufs=4) as al1_pool,
            tc.tile_pool(name="al2", bufs=4) as al2_pool,
            tc.tile_pool(name="srow", bufs=6) as srow_pool,
            tc.tile_pool(name="ge_ps", bufs=2, space=bass.MemorySpace.PSUM) as ge_ps,
            tc.tile_pool(name="p0_ps", bufs=2, space=bass.MemorySpace.PSUM) as p0_ps,
            tc.tile_pool(name="p1_ps", bufs=2, space=bass.MemorySpace.PSUM) as p1_ps,
            tc.tile_pool(name="p2_ps", bufs=2, space=bass.MemorySpace.PSUM) as p2_ps,
        ):
            al_pools = [al0_pool, al1_pool, al2_pool][:NCH]
            ps_pools = [p0_ps, p1_ps, p2_ps][:NCH]
            # ---- constants (warm-up deps first: tag/onesrow/cols/w2/s2) ----
            tag_sb = consts.tile([1, B_LOC * T], bf16)
            nc.sync.dma_start(tag_sb[:], tagrow[:])
            onesrow_b_sb = consts.tile([1, 128], bf16)
            nc.sync.dma_start(onesrow_b_sb[:], onesrow_b[:])
            cols_sb = consts.tile([128, 8], f32)
            nc.sync.dma_start(cols_sb[:], cols[:])
            w2_sb = consts.tile([128, 8, 128], bf16)
            nc.sync.dma_start(w2_sb[:], wq2[:].rearrange("c p k -> p c k"))
            s2_sb = consts.tile([128, 128], bf16)
            nc.sync.dma_start(s2_sb[:], s2[:])
            id_sb = consts.tile([128, 128], f32)
            nc.sync.dma_start(id_sb[:], ident[:])
            ones2_sb = consts.tile([128, 2], bf16)
            nc.sync.dma_start(ones2_sb[:], ones2b[:])

            initcol = cols_sb[:, 0:1]
            bcol = cols_sb[:, 3:4]
            iota = cols_sb[:, 4:5]
            onescol_f = cols_sb[:, 5:6]

            # ---- persistent tiles ----
            # one tile per 64-col block so each GEMM depends only on its DMA
            hts = [
                persist.tile([128, NDC, B_LOC, BT], fp8, name=f"hts{k}")
                for k in range(NB)
            ]
            E2 = persist.tile([128, B_LOC, T], bf16)    # E (rows 64+ reversed)
            emis = persist.tile([128, B_LOC, T], bf16)  # raw emisT+(b-c), rows 0:K
            OH = persist.tile([128, B_LOC, T], bf16)    # one-hot (rows 0:K)
            g1 = persist.tile([128, B_LOC], f32)        # gold emission term
            scr2 = persist.tile([128, T], bf16)         # accum scratch dst
            snap_tiles = [
                persist.tile([128, B_LOC], bf16, name=f"snap{i}")
                for i in range(1, NCH)
            ]
            betas = persist.tile([128, B_LOC], f32)
            wdot = persist.tile([128, B_LOC], f32)

            # ---- stage all hidden data: one DMA per 64-col block ----
            # ONE queue => per-queue in-order completion, so the first
            # blocks land early and the GEMM can stream behind the DMA.
            for k in (0, 7, 1, 6, 2, 5, 3, 4):
                nc.sync.dma_start(hts[k][:], hidtb[k])

            # ---- emissions GEMM: one unit per 64-col block (all seqs) ----
            def unit_blk(k):
                kc = slice(k * BT, (k + 1) * BT)
                rkc = slice((NB - 1 - k) * BT, (NB - k) * BT)
                pe_ = ge_ps.tile([128, B_LOC, BT], f32, tag="ge")
                for dc in range(8):
                    nc.tensor.matmul(
                        pe_[:],
                        w2_sb[:, dc, :],
                        hts[k][:, dc, :, :],
                        start=(dc == 0),
                        stop=(dc == 7),
                    )
                    if dc in (2, 5):
                        yield
                yield
                nc.scalar.activation(
                    E2[0:H2, :, kc], pe_[0:H2, :, :], AF.Exp, bias=bcol[0:H2]
                )
                nc.scalar.activation(
                    E2[H2:128, :, rkc], flip_last(pe_[H2:128, :, :]),
                    AF.Exp, bias=bcol[H2:128],
                )
                yield
                # raw (bias-free) emissions for the gold path; the host adds
                # sum_t b[tag_t] and the T*c shift itself.
                nc.vector.tensor_copy(emis[0:K, :, kc], pe_[0:K, :, :])
                yield

            def unit_goldoh(k):
                # one-hot build for 64-col block k, all seqs
                kc = slice(k * BT, (k + 1) * BT)
                tagap = tag_sb[:].rearrange("p (c t) -> p c t", c=B_LOC)[:, :, kc]
                tb = ge_ps.tile([128, B_LOC, BT], f32, tag="ge")
                nc.tensor.matmul(
                    tb[0:K, :, :], onesrow_b_sb[:, 0:K], tagap,
                    start=True, stop=True,
                )
                yield
                nc.vector.tensor_scalar(
                    OH[0:K, :, kc], tb[0:K, :, :], iota[0:K], None,
                    ALU.is_equal,
                )
                yield

            def unit_goldmul(c):
                nc.gpsimd.tensor_mul(
                    OH[0:K, c, :], emis[0:K, c, :], OH[0:K, c, :]
                )
                yield
                nc.scalar.activation(
                    scr2[0:K, 0:T], OH[0:K, c, :],
                    AF.Identity, accum_out=g1[0:K, c : c + 1],
                )
                yield

            # ---- PE warm-up while the first DMA is in flight ----
            # gold one-hots first (no hidtb dependency), then junk const
            # matmuls: keeps the PE p-state ramping so the real GEMM runs
            # at full clock as soon as block 0 lands.
            for k in range(NB):
                for _ in unit_goldoh(k):
                    pass
            for _ in range(4):
                jk_ps = p0_ps.tile([128, B_LOC], f32, tag="s0", name="jk")
                nc.tensor.matmul(
                    jk_ps[:], s2_sb[:], s2_sb[:, 0:B_LOC],
                    start=True, stop=True,
                )

            # ---- pre-scan: all GEMM blocks ----
            for k in (0, 7, 1, 6, 2, 5, 3, 4):
                for _ in unit_blk(k):
                    pass

            # ---- chain inits ----
            # NCH merged fwd/bwd chains over cols 1..255; chain i>0 warm-
            # starts WARMN cols before its cut with state := an E2 column.
            cuts = [(255 * i) // NCH for i in range(NCH + 1)]  # e.g. 0,85,170,255
            als = []
            for i in range(NCH):
                al = al_pools[i].tile([128, B_LOC], bf16, tag=f"a{i}",
                                      name=f"al_i{i}")
                if i == 0:
                    nc.vector.tensor_scalar_mul(al[:], E2[:, :, 0], initcol)
                else:
                    nc.vector.tensor_copy(al[:], E2[:, :, cuts[i] - WARMN])
                als.append(al)

            # gold multiplies/accums pumped into scan gaps
            work = [unit_goldmul(c) for c in range(B_LOC)]

            def pump(n):
                for _ in range(n):
                    while work:
                        try:
                            next(work[0])
                            break
                        except StopIteration:
                            work.pop(0)

            if pump_mode == 0:
                pump(len(work) * 16)

            # ---- merged fwd/bwd chains, interleaved ----
            snaps = [None] + snap_tiles
            finals = [None] * NCH
            maxsteps = max(cuts[i + 1] - (cuts[i] - WARMN if i else 0)
                           for i in range(NCH))
            for j in range(1, maxsteps + 1):
                for i in range(NCH):
                    col = (cuts[i] - WARMN if i else 0) + j
                    if col > cuts[i + 1]:
                        continue
                    psi = ps_pools[i].tile([128, B_LOC], f32, tag=f"s{i}",
                                           name=f"ps_i{i}")
                    nc.tensor.matmul(psi[:], s2_sb[:], als[i][:],
                                     start=True, stop=True)
                    al_new = al_pools[i].tile([128, B_LOC], bf16, tag=f"a{i}",
                                              name=f"aln_i{i}")
                    nc.vector.tensor_mul(al_new[:], psi[:], E2[:, :, col])
                    als[i] = al_new
                    if i > 0 and col == cuts[i]:  # handoff snapshot
                        nc.vector.tensor_copy(snaps[i][:], al_new[:])
                    if col == cuts[i + 1]:
                        finals[i] = al_new
                if pump_mode:
                    pump(pump_mode)

            pump(len(work) * 16)  # drain remaining background work

            # ---- finisher ----
            # z = alpha^_255 . (A gamma^_256) on the last chain's final state
            alf = finals[NCH - 1]
            ps_f = p0_ps.tile([128, B_LOC], f32, tag="s0", name="psf")
            nc.tensor.matmul(ps_f[:], s2_sb[:], alf[:], start=True, stop=True)
            nc.vector.tensor_copy(betas[H2 : H2 + K, :], ps_f[H2 : H2 + K, :])
            psz = p0_ps.tile([128, B_LOC], f32, tag="s0", name="psz")
            nc.tensor.matmul(
                psz[0:K, :], id_sb[H2 : H2 + K, H2 : H2 + K],
                betas[H2 : H2 + K, :], start=True, stop=True,
            )
            nc.vector.tensor_mul(wdot[0:K, :], psz[0:K, :], alf[0:K, :])
            zz = p0_ps.tile([128, B_LOC], f32, tag="s0", name="zz")
            nc.tensor.matmul(zz[0:1, :], onescol_f[0:K], wdot[0:K, :],
                             start=True, stop=True)
            lnz = srow_pool.tile([1, B_LOC], f32, tag="srow")
            nc.scalar.activation(lnz[:], zz[0:1, :], AF.Ln)

            # telescoping scale-ratio corrections across the NCH-1 handoffs
            def lnsum2(state_bf16, nm):
                ps_r = p1_ps.tile([128, B_LOC], f32, tag="s1", name=f"pr{nm}")
                nc.tensor.matmul(ps_r[0:2, :], ones2_sb[:], state_bf16,
                                 start=True, stop=True)
                lt = srow_pool.tile([2, B_LOC], f32, tag="srow", name=f"lt{nm}")
                nc.scalar.activation(lt[:], ps_r[0:2, :], AF.Ln)
                ps_s = p1_ps.tile([128, B_LOC], f32, tag="s1", name=f"pss{nm}")
                nc.tensor.matmul(ps_s[0:1, :], onescol_f[0:2], lt[:],
                                 start=True, stop=True)
                row = srow_pool.tile([1, B_LOC], f32, tag="srow", name=f"row{nm}")
                nc.vector.tensor_copy(row[:], ps_s[0:1, :])
                return row

            lnp_true = [lnsum2(finals[i][:], f"t{i}") for i in range(NCH - 1)]
            lnp_hat = [lnsum2(snaps[i][:], f"h{i}") for i in range(1, NCH)]

            # gold total (emission term only; host adds transition score)
            gzz = p2_ps.tile([128, B_LOC], f32, tag="s2", name="gzz")
            nc.tensor.matmul(gzz[0:1, :], onescol_f[0:K], g1[0:K, :],
                             start=True, stop=True)

            outrow = srow_pool.tile([1, B_LOC], f32, tag="srow")
            nc.vector.tensor_sub(outrow[:], lnz[:], gzz[0:1, :])
            for i in range(NCH - 1):
                nc.vector.tensor_add(outrow[:], outrow[:], lnp_true[i][:])
                nc.vector.tensor_sub(outrow[:], outrow[:], lnp_hat[i][:])
            nc.sync.dma_start(out_d[:], outrow[:])

    nc.compile()
    return nc


def _get_compiled(dbg=False):
    key = ("dbg" if dbg else "nc")
    if key not in _COMPILED:
        _COMPILED[key] = _build(dbg)
    return _COMPILED[key]


def _host_inputs(W, b, transitions, start_trans, end_trans):
    import ml_dtypes

    bf16 = ml_dtypes.bfloat16
    expA = np.exp(transitions).astype(np.float32)
    s2 = np.zeros((128, 128), np.float32)
    s2[0:K, 0:K] = expA
    s2[H2 : H2 + K, H2 : H2 + K] = expA.T

    wq2 = np.zeros((8, 128, 128), np.float32)
    wr = W.reshape(8, 128, K)
    wq2[:, :, 0:K] = wr
    wq2[:, :, H2 : H2 + K] = wr

    # growth-neutralising constant: E[colsum of exp(emis+b)] for h ~ N(0, I)
    c_shift = float(
        np.log(np.sum(np.exp(b.astype(np.float64)
                             + 0.5 * np.sum(W.astype(np.float64) ** 2, axis=0))))
    )

    cols = np.zeros((128, 8), np.float32)
    cols[0:K, 0] = np.exp(start_trans)
    cols[H2 : H2 + K, 0] = np.exp(end_trans)
    cols[0:K, 3] = b - c_shift
    cols[H2 : H2 + K, 3] = b - c_shift
    cols[0:K, 4] = np.arange(K, dtype=np.float32)
    cols[0:K, 5] = 1.0

    ones2 = np.zeros((128, 2), np.float32)
    ones2[0:K, 0] = 1.0
    ones2[H2 : H2 + K, 1] = 1.0

    common = {
        "wq2": np.ascontiguousarray(wq2.astype(bf16)),
        "s2": np.ascontiguousarray(s2.astype(bf16)),
        "ident": np.eye(128, dtype=np.float32),
        "cols": np.ascontiguousarray(cols),
        "ones2b": np.ascontiguousarray(ones2.astype(bf16)),
        "onesrow_b": np.ones((1, 128), bf16),
    }
    return common, c_shift


def kernel(full_hidden, tag_ids, mask, W, b, transitions, start_trans, end_trans,
           dbg=False):
    global LAST_RESULT
    import ml_dtypes
    from concourse.bass_utils import run_bass_kernel_spmd

    bf16 = ml_dtypes.bfloat16
    full_hidden = np.asarray(full_hidden, dtype=np.float32)
    tags = np.asarray(tag_ids)
    W = np.asarray(W, dtype=np.float32)
    b = np.asarray(b, dtype=np.float32)
    transitions = np.asarray(transitions, dtype=np.float32)
    start_trans = np.asarray(start_trans, dtype=np.float32)
    end_trans = np.asarray(end_trans, dtype=np.float32)

    nc = _get_compiled(dbg)
    common, c_shift = _host_inputs(W, b, transitions, start_trans, end_trans)

    # pre-transposed + blocked hidden, block-major and partition-major
    hb = full_hidden.astype(ml_dtypes.float8_e4m3fn)  # [B, T, D]
    hbt = hb.transpose(0, 2, 1).reshape(B_FULL, NDC, 128, NB, BT)
    hbt = hbt.transpose(3, 2, 1, 0, 4)               # [NB, 128, NDC, B, BT]

    in_maps = []
    for c in range(N_CORES):
        sl = slice(c * B_LOC, (c + 1) * B_LOC)
        in_maps.append(
            {
                "hidtb": np.ascontiguousarray(hbt[:, :, :, sl, :]),  # [NB,128,NDC,B,BT]
                "tagrow": np.ascontiguousarray(
                    tags[sl].astype(np.float32).reshape(1, B_LOC * T).astype(bf16)
                ),
                **common,
            }
        )

    # host-side gold transition/start/end score (depends only on tag_ids/mask)
    m = np.asarray(mask).astype(bool)
    tg = tags.astype(np.int64)
    first = tg[:, 0]
    tscore = start_trans[first].astype(np.float64)
    prev = first.copy()
    for t in range(1, T):
        step = transitions[prev, tg[:, t]]
        tscore = np.where(m[:, t], tscore + step, tscore)
        prev = np.where(m[:, t], tg[:, t], prev)
    tscore = tscore + end_trans[prev]
    # device gold accumulates RAW emissions (no bias, no -c shift), while
    # ln z carries -T*c from the E bias; reconcile both here.
    tscore = tscore + b.astype(np.float64)[tg].sum(axis=1) - T * c_shift

    res = run_bass_kernel_spmd(nc, in_maps, core_ids=list(range(N_CORES)))
    LAST_RESULT = res
    out = np.concatenate(
        [np.asarray(res.results[c]["out"]).reshape(B_LOC) for c in range(N_CORES)]
    )
    # -c_shift bias cancels between ln z (-T*c) and the gold accumulator.
    return (out.astype(np.float64) - tscore).astype(np.float32)


# revision 44
# speedup vs baseline: 1.3816x; 1.1460x over previous
"""CRF negative-log-likelihood kernel for Trainium2, SPMD over 8 NeuronCores.

v5 strategy
-----------
Data-parallel over batch: core c handles sequences b in [c*8, (c+1)*8).

Per core (B=8 local sequences, T=512, K=50 tags, D=1024):

1. Hidden states arrive PRE-TRANSPOSED and BLOCKED from the host as
   hidtb[block, dchunk, 128d, seq, 64t] bf16 -- one contiguous 1MB DMA
   per 64-column block (8 DMAs, 2 queues). No device transposes.

2. Emissions GEMM (bf16): per 64-col block (all 8 seqs at once):
   8 accumulating d-chunk matmuls with 512-wide moving [8 seq x 64 t]
   into one PSUM bank; column-DOUBLED stationary W2 puts emisT on both
   partition row-blocks. Act-exp with bias (b - c) produces E, where
   c = ln sum_k exp(b_k + ||W_k||^2/2) keeps the recurrence
   growth-neutral with NO per-column normalisation (exact math:
   the -c factors cancel between ln Z and the gold emission term).

3. E storage: rows 0:64 hold E_t at column t, rows 64:128 hold
   E_{511-t} at column t (time reversed), so one merged fwd/bwd chain
   step reads a single column.

4. TWO merged chains run concurrently (interleaved on PE+DVE):
     chain 1: cols 1..127   (fwd t=1..127   / bwd t=510..384)
     chain 2: cols 121..255 (fwd t=121..255 / bwd t=390..256),
              warm-started at col 120 with state := E2[:, :, 120].
   The CRF step matrix mixes with contraction ~0.03/step, so after
   chain 2's 7 warm-up steps its state direction is exact to ~1e-11;
   the unknown warm-start SCALE is removed exactly by the ratio
     (1.alpha_127)(1.gamma_384) / (1.alpha^_127)(1.gamma^_384)
   using chain 1's final state and a snapshot of chain 2 at col 127.
   log Z = ln(alpha^_255 . A gamma^_256) + ln-ratio  (+ T*c, which
   cancels against the gold accumulator).  ~136 rounds instead of 255.

5. Gold path: device computes only sum_t emis[tag_t, t] (one-hot via
   rank-1 tag broadcast + iota-compare in prescan; GpSimd multiply +
   Scalar accumulate pumped into scan gaps). Transition/start/end gold
   score is computed on the host from tag_ids alone.
"""

import numpy as np

B_FULL = 64
B_LOC = 8
T = 512
K = 50
D = 1024
N_CORES = 8
H2 = 64   # partition base of the bwd/second row block
NDC = D // 128  # 8 d-chunks
NB = 8    # t-blocks
BT = T // NB    # 64 cols per block
NCH = 8         # concurrent merged chains (share one matmul per round)
WARMN = 7       # warm-up steps per handoff
NR = (255 + WARMN * (NCH - 1)) // NCH  # 38 rounds; chain i covers cols 31i..31i+38
STRIDE = NR - WARMN                     # 31: start_i = STRIDE * i

_COMPILED = {}
LAST_RESULT = None


def _build(dbg=False):
    import os

    import concourse.bass as bass
    import concourse.tile as tile
    from concourse import bacc, mybir

    pump_mode = int(os.environ.get("V2_PUMP", "2"))  # 0=no interleave

    f32 = mybir.dt.float32
    bf16 = mybir.dt.bfloat16
    fp8 = mybir.dt.float8e4

    nc = bacc.Bacc(
        "TRN2",
        target_bir_lowering=False,
        debug=False,
        num_devices=N_CORES,
    )

    def flip_last(ap):
        """Reverse the innermost free dim of an AP (negative stride)."""
        st, n = ap.ap[-1]
        return bass.AP(ap.tensor, ap.offset + (n - 1) * st,
                       ap.ap[:-1] + [[-st, n]])

    hidtb = nc.dram_tensor("hidtb", [NB, 128, NDC, B_LOC, BT], fp8,
                           kind="ExternalInput")
    wq2 = nc.dram_tensor("wq2", [8, 128, 128], bf16, kind="ExternalInput")
    s2 = nc.dram_tensor("s2", [128, 128], bf16, kind="ExternalInput")
    ident = nc.dram_tensor("ident", [128, 128], f32, kind="ExternalInput")
    # cols: 0=initcol(exp st | exp en) 3=bcol(b-c) 4=iota 5=ones(0:K)
    cols = nc.dram_tensor("cols", [128, 8], f32, kind="ExternalInput")
    # ones2b: col0 = ones on rows 0:K, col1 = ones on rows H2:H2+K (bf16)
    ones2b = nc.dram_tensor("ones2b", [128, 2], bf16, kind="ExternalInput")
    onesrow_b = nc.dram_tensor("onesrow_b", [1, 128], bf16, kind="ExternalInput")
    tagrow = nc.dram_tensor("tagrow", [1, B_LOC * T], bf16, kind="ExternalInput")
    out_d = nc.dram_tensor("out", [1, B_LOC], f32, kind="ExternalOutput")
    if dbg:
        dbg_st = nc.dram_tensor("dbg_st", [128, 4 * B_LOC], f32,
                                kind="ExternalOutput")

    AF = mybir.ActivationFunctionType
    ALU = mybir.AluOpType

    with tile.TileContext(nc) as tc:
        with (
            tc.tile_pool(name="consts", bufs=1) as consts,
            tc.tile_pool(name="persist", bufs=1) as persist,
            tc.tile_pool(name="alp", bufs=4) as alp_pool,
            tc.tile_pool(name="srow", bufs=6) as srow_pool,
            tc.tile_pool(name="ge_ps", bufs=3, space=bass.MemorySpace.PSUM) as ge_ps,
            tc.tile_pool(name="sc_ps", bufs=3, space=bass.MemorySpace.PSUM) as sc_ps,
        ):
            # ---- constants (warm-up deps first: tag/onesrow/cols/w2/s2) ----
            tag_sb = consts.tile([1, B_LOC * T], bf16)
            nc.sync.dma_start(tag_sb[:], tagrow[:])
            onesrow_b_sb = consts.tile([1, 128], bf16)
            nc.sync.dma_start(onesrow_b_sb[:], onesrow_b[:])
            cols_sb = consts.tile([128, 8], f32)
            nc.sync.dma_start(cols_sb[:], cols[:])
            w2_sb = consts.tile([128, 8, 128], bf16)
            nc.sync.dma_start(w2_sb[:], wq2[:].rearrange("c p k -> p c k"))
            s2_sb = consts.tile([128, 128], bf16)
            nc.sync.dma_start(s2_sb[:], s2[:])
            id_sb = consts.tile([128, 128], f32)
            nc.sync.dma_start(id_sb[:], ident[:])
            ones2_sb = consts.tile([128, 2], bf16)
            nc.sync.dma_start(ones2_sb[:], ones2b[:])

            initcol = cols_sb[:, 0:1]
            bcol = cols_sb[:, 3:4]
            iota = cols_sb[:, 4:5]
            onescol_f = cols_sb[:, 5:6]

            # ---- persistent tiles ----
            # one tile per 64-col block so each GEMM depends only on its DMA
            hts = [
                persist.tile([128, NDC, B_LOC, BT], fp8, name=f"hts{k}")
                for k in range(NB)
            ]
            # E interleaved by (round j, chain i, seq): slot (j,i) = col 31i+j
            E2I = persist.tile([128, NR + 1, NCH, B_LOC], bf16)
            emis = persist.tile([128, B_LOC, T], bf16)  # raw emisT+(b-c), rows 0:K
            OH = persist.tile([128, B_LOC, T], bf16)    # one-hot (rows 0:K)
            g1 = persist.tile([128, B_LOC], f32)        # gold emission term
            scr2 = persist.tile([128, T], bf16)         # accum scratch dst
            snapT = persist.tile([128, NCH - 1, B_LOC], bf16)
            betas = persist.tile([128, B_LOC], f32)
            wdot = persist.tile([128, B_LOC], f32)

            # ---- stage all hidden data: one DMA per 64-col block ----
            # ONE queue => per-queue in-order completion, so the first
            # blocks land early and the GEMM can stream behind the DMA.
            for k in (0, 7, 1, 6, 2, 5, 3, 4):
                nc.sync.dma_start(hts[k][:], hidtb[k])

            # ---- emissions GEMM: one unit per 64-col block (all seqs) ----
            def swap2(ap):
                """Swap the last two free-dim levels of an AP."""
                return bass.AP(ap.tensor, ap.offset,
                               ap.ap[:-2] + [ap.ap[-1], ap.ap[-2]])

            def flipd(ap, lvl):
                """Reverse free-dim level lvl (negative index) of an AP."""
                lv = list(ap.ap)
                st, n = lv[lvl]
                off = ap.offset + (n - 1) * st
                lv[lvl] = [-st, n]
                return bass.AP(ap.tensor, off, lv)

            def unit_blk(k):
                kc = slice(k * BT, (k + 1) * BT)
                pe_ = ge_ps.tile([128, B_LOC, BT], f32, tag="ge")
                for dc in range(8):
                    nc.tensor.matmul(
                        pe_[:],
                        w2_sb[:, dc, :],
                        hts[k][:, dc, :, :],
                        start=(dc == 0),
                        stop=(dc == 7),
                    )
                    if dc in (2, 5):
                        yield
                yield
                # fwd rows: slot (j,i) <- E(col 31i+j) for cols in this block
                for i in range(NCH):
                    ja = max(0, k * BT - STRIDE * i)
                    jb = min(NR, k * BT + BT - 1 - STRIDE * i)
                    if ja > jb:
                        continue
                    a = STRIDE * i + ja - k * BT
                    n = jb - ja + 1
                    nc.scalar.activation(
                        E2I[0:H2, ja : jb + 1, i, :],
                        swap2(pe_[0:H2, :, a : a + n]),
                        AF.Exp, bias=bcol[0:H2],
                    )
                    yield
                # bwd rows: slot (j,i) <- E(col 511-31i-j)
                for i in range(NCH):
                    ja = max(0, (T - BT - 64 * k) - STRIDE * i)
                    jb = min(NR, (T - 1 - 64 * k) - STRIDE * i)
                    if ja > jb:
                        continue
                    n = jb - ja + 1
                    a0 = (T - 1 - 64 * k) - STRIDE * i - jb
                    nc.scalar.activation(
                        E2I[H2:128, ja : jb + 1, i, :],
                        flipd(swap2(pe_[H2:128, :, a0 : a0 + n]), -2),
                        AF.Exp, bias=bcol[H2:128],
                    )
                    yield
                # raw (bias-free) emissions for the gold path; the host adds
                # sum_t b[tag_t] and the T*c shift itself.
                nc.vector.tensor_copy(emis[0:K, :, kc], pe_[0:K, :, :])
                yield

            def unit_goldoh(k):
                # one-hot build for 64-col block k, all seqs
                kc = slice(k * BT, (k + 1) * BT)
                tagap = tag_sb[:].rearrange("p (c t) -> p c t", c=B_LOC)[:, :, kc]
                tb = ge_ps.tile([128, B_LOC, BT], f32, tag="ge")
                nc.tensor.matmul(
                    tb[0:K, :, :], onesrow_b_sb[:, 0:K], tagap,
                    start=True, stop=True,
                )
                yield
                nc.vector.tensor_scalar(
                    OH[0:K, :, kc], tb[0:K, :, :], iota[0:K], None,
                    ALU.is_equal,
                )
                yield

            def unit_goldmul(c):
                nc.gpsimd.tensor_mul(
                    OH[0:K, c, :], emis[0:K, c, :], OH[0:K, c, :]
                )
                yield
                nc.scalar.activation(
                    scr2[0:K, 0:T], OH[0:K, c, :],
                    AF.Identity, accum_out=g1[0:K, c : c + 1],
                )
                yield

            # ---- PE warm-up while the first DMA is in flight ----
            # gold one-hots first (no hidtb dependency), then junk const
            # matmuls: keeps the PE p-state ramping so the real GEMM runs
            # at full clock as soon as block 0 lands.
            for k in range(NB):
                for _ in unit_goldoh(k):
                    pass
            for _ in range(4):
                jk_ps = sc_ps.tile([128, B_LOC], f32, tag="sc", name="jk")
                nc.tensor.matmul(
                    jk_ps[:], s2_sb[:], s2_sb[:, 0:B_LOC],
                    start=True, stop=True,
                )

            # ---- pre-scan: all GEMM blocks ----
            for k in (0, 7, 1, 6, 2, 5, 3, 4):
                for _ in unit_blk(k):
                    pass

            # ---- chain inits: alpha[:, i, :] = E2I slot (0, i); chain 0
            # additionally multiplied by exp(start)/exp(end).
            alpha = alp_pool.tile([128, NCH, B_LOC], bf16, tag="al")
            nc.vector.tensor_copy(alpha[:, 1:, :], E2I[:, 0, 1:, :])
            nc.vector.tensor_scalar_mul(alpha[:, 0, :], E2I[:, 0, 0, :], initcol)

            # gold multiplies/accums pumped into scan gaps
            work = [unit_goldmul(c) for c in range(B_LOC)]

            def pump(n):
                for _ in range(n):
                    while work:
                        try:
                            next(work[0])
                            break
                        except StopIteration:
                            work.pop(0)

            if pump_mode == 0:
                pump(len(work) * 16)

            # ---- all chains advance with ONE matmul + ONE multiply/round --
            for j in range(1, NR + 1):
                psj = sc_ps.tile([128, NCH, B_LOC], f32, tag="sc", name="psj")
                nc.tensor.matmul(psj[:], s2_sb[:], alpha[:],
                                 start=True, stop=True)
                alpha_new = alp_pool.tile([128, NCH, B_LOC], bf16, tag="al",
                                          name="aln")
                nc.vector.tensor_mul(alpha_new[:], psj[:], E2I[:, j, :, :])
                alpha = alpha_new
                if j == WARMN:  # all handoff snapshots in one copy
                    nc.vector.tensor_copy(snapT[:], alpha[:, 1:, :])
                if pump_mode:
                    pump(pump_mode)

            pump(len(work) * 16)  # drain remaining background work

            # ---- finisher ----
            # z = alpha^_255 . (A gamma^_256) on the last chain's final state
            alf = alpha[:, NCH - 1, :]
            ps_f = sc_ps.tile([128, B_LOC], f32, tag="sc", name="psf")
            nc.tensor.matmul(ps_f[:], s2_sb[:], alf, start=True, stop=True)
            nc.vector.tensor_copy(betas[H2 : H2 + K, :], ps_f[H2 : H2 + K, :])
            psz = sc_ps.tile([128, B_LOC], f32, tag="sc", name="psz")
            nc.tensor.matmul(
                psz[0:K, :], id_sb[H2 : H2 + K, H2 : H2 + K],
                betas[H2 : H2 + K, :], start=True, stop=True,
            )
            nc.vector.tensor_mul(wdot[0:K, :], psz[0:K, :], alf[0:K, :])
            zz = sc_ps.tile([128, B_LOC], f32, tag="sc", name="zz")
            nc.tensor.matmul(zz[0:1, :], onescol_f[0:K], wdot[0:K, :],
                             start=True, stop=True)
            lnz = srow_pool.tile([1, B_LOC], f32, tag="srow")
            nc.scalar.activation(lnz[:], zz[0:1, :], AF.Ln)

            # telescoping scale ratios: sum_i [ln S(final_i) - ln S(snap_i+1)]
            # finals of chains 0..NCH-2 are alpha[:, 0:NCH-1, :].
            NH = NCH - 1
            pr_t = sc_ps.tile([128, NH * B_LOC], f32, tag="sc", name="prt")
            nc.tensor.matmul(pr_t[0:2, :], ones2_sb[:],
                             alpha[:, 0:NH, :], start=True, stop=True)
            pr_h = sc_ps.tile([128, NH * B_LOC], f32, tag="sc", name="prh")
            nc.tensor.matmul(pr_h[0:2, :], ones2_sb[:], snapT[:],
                             start=True, stop=True)
            lt_t = srow_pool.tile([2, NH, B_LOC], f32, tag="srow", name="ltt")
            nc.scalar.activation(lt_t[:], pr_t[0:2, :], AF.Ln)
            lt_h = srow_pool.tile([2, NH, B_LOC], f32, tag="srow", name="lth")
            nc.scalar.activation(lt_h[:], pr_h[0:2, :], AF.Ln)
            nc.vector.tensor_sub(lt_t[:], lt_t[:], lt_h[:])
            ps_s = sc_ps.tile([128, NH * B_LOC], f32, tag="sc", name="pss")
            nc.tensor.matmul(ps_s[0:1, :], onescol_f[0:2], lt_t[:],
                             start=True, stop=True)
            lnr = srow_pool.tile([1, NH, B_LOC], f32, tag="srow", name="lnr")
            nc.vector.tensor_copy(lnr[:], ps_s[0:1, :])

            # gold total (emission term only; host adds transition score)
            gzz = sc_ps.tile([128, B_LOC], f32, tag="sc", name="gzz")
            nc.tensor.matmul(gzz[0:1, :], onescol_f[0:K], g1[0:K, :],
                             start=True, stop=True)

            outrow = srow_pool.tile([1, B_LOC], f32, tag="srow")
            nc.vector.tensor_sub(outrow[:], lnz[:], gzz[0:1, :])
            for i in range(NCH - 1):
                nc.vector.tensor_add(outrow[:], outrow[:], lnr[:, i, :])
            nc.sync.dma_start(out_d[:], outrow[:])

    nc.compile()
    return nc


def _get_compiled(dbg=False):
    key = ("dbg" if dbg else "nc")
    if key not in _COMPILED:
        _COMPILED[key] = _build(dbg)
    return _COMPILED[key]


def _host_inputs(W, b, transitions, start_trans, end_trans):
    import ml_dtypes

    bf16 = ml_dtypes.bfloat16
    expA = np.exp(transitions).astype(np.float32)
    s2 = np.zeros((128, 128), np.float32)
    s2[0:K, 0:K] = expA
    s2[H2 : H2 + K, H2 : H2 + K] = expA.T

    wq2 = np.zeros((8, 128, 128), np.float32)
    wr = W.reshape(8, 128, K)
    wq2[:, :, 0:K] = wr
    wq2[:, :, H2 : H2 + K] = wr

    # growth-neutralising constant: E[colsum of exp(emis+b)] for h ~ N(0, I)
    c_shift = float(
        np.log(np.sum(np.exp(b.astype(np.float64)
                             + 0.5 * np.sum(W.astype(np.float64) ** 2, axis=0))))
    )

    cols = np.zeros((128, 8), np.float32)
    cols[0:K, 0] = np.exp(start_trans)
    cols[H2 : H2 + K, 0] = np.exp(end_trans)
    cols[0:K, 3] = b - c_shift
    cols[H2 : H2 + K, 3] = b - c_shift
    cols[0:K, 4] = np.arange(K, dtype=np.float32)
    cols[0:K, 5] = 1.0

    ones2 = np.zeros((128, 2), np.float32)
    ones2[0:K, 0] = 1.0
    ones2[H2 : H2 + K, 1] = 1.0

    common = {
        "wq2": np.ascontiguousarray(wq2.astype(bf16)),
        "s2": np.ascontiguousarray(s2.astype(bf16)),
        "ident": np.eye(128, dtype=np.float32),
        "cols": np.ascontiguousarray(cols),
        "ones2b": np.ascontiguousarray(ones2.astype(bf16)),
        "onesrow_b": np.ones((1, 128), bf16),
    }
    return common, c_shift


def kernel(full_hidden, tag_ids, mask, W, b, transitions, start_trans, end_trans,
           dbg=False):
    global LAST_RESULT
    import ml_dtypes
    from concourse.bass_utils import run_bass_kernel_spmd

    bf16 = ml_dtypes.bfloat16
    full_hidden = np.asarray(full_hidden, dtype=np.float32)
    tags = np.asarray(tag_ids)
    W = np.asarray(W, dtype=np.float32)
    b = np.asarray(b, dtype=np.float32)
    transitions = np.asarray(transitions, dtype=np.float32)
    start_trans = np.asarray(start_trans, dtype=np.float32)
    end_trans = np.asarray(end_trans, dtype=np.float32)

    nc = _get_compiled(dbg)
    common, c_shift = _host_inputs(W, b, transitions, start_trans, end_trans)

    # pre-transposed + blocked hidden, block-major and partition-major
    hb = full_hidden.astype(ml_dtypes.float8_e4m3fn)  # [B, T, D]
    hbt = hb.transpose(0, 2, 1).reshape(B_FULL, NDC, 128, NB, BT)
    hbt = hbt.transpose(3, 2, 1, 0, 4)               # [NB, 128, NDC, B, BT]

    in_maps = []
    for c in range(N_CORES):
        sl = slice(c * B_LOC, (c + 1) * B_LOC)
        in_maps.append(
            {
                "hidtb": np.ascontiguousarray(hbt[:, :, :, sl, :]),  # [NB,128,NDC,B,BT]
                "tagrow": np.ascontiguousarray(
                    tags[sl].astype(np.float32).reshape(1, B_LOC * T).astype(bf16)
                ),
                **common,
            }
        )

    # host-side gold transition/start/end score (depends only on tag_ids/mask)
    m = np.asarray(mask).astype(bool)
    tg = tags.astype(np.int64)
    first = tg[:, 0]
    tscore = start_trans[first].astype(np.float64)
    prev = first.copy()
    for t in range(1, T):
        step = transitions[prev, tg[:, t]]
        tscore = np.where(m[:, t], tscore + step, tscore)
        prev = np.where(m[:, t], tg[:, t], prev)
    tscore = tscore + end_trans[prev]
    # device gold accumulates RAW emissions (no bias, no -c shift), while
    # ln z carries -T*c from the E bias; reconcile both here.
    tscore = tscore + b.astype(np.float64)[tg].sum(axis=1) - T * c_shift

    res = run_bass_kernel_spmd(nc, in_maps, core_ids=list(range(N_CORES)))
    LAST_RESULT = res
    out = np.concatenate(
        [np.asarray(res.results[c]["out"]).reshape(B_LOC) for c in range(N_CORES)]
    )
    # -c_shift bias cancels between ln z (-T*c) and the gold accumulator.
    return (out.astype(np.float64) - tscore).astype(np.float32)


# revision 45
# speedup vs baseline: 1.6125x; 1.1671x over previous
"""CRF negative-log-likelihood kernel for Trainium2, SPMD over 8 NeuronCores.

v5 strategy
-----------
Data-parallel over batch: core c handles sequences b in [c*8, (c+1)*8).

Per core (B=8 local sequences, T=512, K=50 tags, D=1024):

1. Hidden states arrive PRE-TRANSPOSED and BLOCKED from the host as
   hidtb[block, dchunk, 128d, seq, 64t] bf16 -- one contiguous 1MB DMA
   per 64-column block (8 DMAs, 2 queues). No device transposes.

2. Emissions GEMM (bf16): per 64-col block (all 8 seqs at once):
   8 accumulating d-chunk matmuls with 512-wide moving [8 seq x 64 t]
   into one PSUM bank; column-DOUBLED stationary W2 puts emisT on both
   partition row-blocks. Act-exp with bias (b - c) produces E, where
   c = ln sum_k exp(b_k + ||W_k||^2/2) keeps the recurrence
   growth-neutral with NO per-column normalisation (exact math:
   the -c factors cancel between ln Z and the gold emission term).

3. E storage: rows 0:64 hold E_t at column t, rows 64:128 hold
   E_{511-t} at column t (time reversed), so one merged fwd/bwd chain
   step reads a single column.

4. TWO merged chains run concurrently (interleaved on PE+DVE):
     chain 1: cols 1..127   (fwd t=1..127   / bwd t=510..384)
     chain 2: cols 121..255 (fwd t=121..255 / bwd t=390..256),
              warm-started at col 120 with state := E2[:, :, 120].
   The CRF step matrix mixes with contraction ~0.03/step, so after
   chain 2's 7 warm-up steps its state direction is exact to ~1e-11;
   the unknown warm-start SCALE is removed exactly by the ratio
     (1.alpha_127)(1.gamma_384) / (1.alpha^_127)(1.gamma^_384)
   using chain 1's final state and a snapshot of chain 2 at col 127.
   log Z = ln(alpha^_255 . A gamma^_256) + ln-ratio  (+ T*c, which
   cancels against the gold accumulator).  ~136 rounds instead of 255.

5. Gold path: device computes only sum_t emis[tag_t, t] (one-hot via
   rank-1 tag broadcast + iota-compare in prescan; GpSimd multiply +
   Scalar accumulate pumped into scan gaps). Transition/start/end gold
   score is computed on the host from tag_ids alone.
"""

import numpy as np

B_FULL = 64
B_LOC = 8
T = 512
K = 50
D = 1024
N_CORES = 8
H2 = 64   # partition base of the bwd/second row block
NDC = D // 128  # 8 d-chunks
NB = 8    # t-blocks
BT = T // NB    # 64 cols per block
NCH = 8         # concurrent merged chains (share one matmul per round)
WARMN = 7       # warm-up steps per handoff
NR = (255 + WARMN * (NCH - 1)) // NCH  # 38 rounds; chain i covers cols 31i..31i+38
STRIDE = NR - WARMN                     # 31: start_i = STRIDE * i

_COMPILED = {}
LAST_RESULT = None


def _build(dbg=False):
    import os

    import concourse.bass as bass
    import concourse.tile as tile
    from concourse import bacc, mybir

    pump_mode = int(os.environ.get("V2_PUMP", "2"))  # 0=no interleave

    f32 = mybir.dt.float32
    bf16 = mybir.dt.bfloat16
    fp8 = mybir.dt.float8e4

    nc = bacc.Bacc(
        "TRN2",
        target_bir_lowering=False,
        debug=False,
        num_devices=N_CORES,
    )

    def flip_last(ap):
        """Reverse the innermost free dim of an AP (negative stride)."""
        st, n = ap.ap[-1]
        return bass.AP(ap.tensor, ap.offset + (n - 1) * st,
                       ap.ap[:-1] + [[-st, n]])

    hidtb = nc.dram_tensor("hidtb", [NB, 128, NDC, B_LOC, BT], fp8,
                           kind="ExternalInput")
    wq2 = nc.dram_tensor("wq2", [8, 128, 128], bf16, kind="ExternalInput")
    s2 = nc.dram_tensor("s2", [128, 128], bf16, kind="ExternalInput")
    ident = nc.dram_tensor("ident", [128, 128], f32, kind="ExternalInput")
    # cols: 0=initcol(exp st | exp en) 3=bcol(b-c) 4=iota 5=ones(0:K)
    cols = nc.dram_tensor("cols", [128, 8], f32, kind="ExternalInput")
    # ones2b: col0 = ones on rows 0:K, col1 = ones on rows H2:H2+K (bf16)
    ones2b = nc.dram_tensor("ones2b", [128, 2], bf16, kind="ExternalInput")
    onesrow_b = nc.dram_tensor("onesrow_b", [1, 128], bf16, kind="ExternalInput")
    tagrow = nc.dram_tensor("tagrow", [1, B_LOC * T], bf16, kind="ExternalInput")
    out_d = nc.dram_tensor("out", [1, B_LOC], f32, kind="ExternalOutput")
    if dbg:
        dbg_st = nc.dram_tensor("dbg_st", [128, 4 * B_LOC], f32,
                                kind="ExternalOutput")

    AF = mybir.ActivationFunctionType
    ALU = mybir.AluOpType

    with tile.TileContext(nc) as tc:
        with (
            tc.tile_pool(name="consts", bufs=1) as consts,
            tc.tile_pool(name="persist", bufs=1) as persist,
            tc.tile_pool(name="alp", bufs=4) as alp_pool,
            tc.tile_pool(name="srow", bufs=6) as srow_pool,
            tc.tile_pool(name="ge_ps", bufs=3, space=bass.MemorySpace.PSUM) as ge_ps,
            tc.tile_pool(name="sc_ps", bufs=3, space=bass.MemorySpace.PSUM) as sc_ps,
        ):
            # ---- constants (warm-up deps first: tag/onesrow/cols/w2/s2) ----
            tag_sb = consts.tile([1, B_LOC * T], bf16)
            nc.sync.dma_start(tag_sb[:], tagrow[:])
            onesrow_b_sb = consts.tile([1, 128], bf16)
            nc.sync.dma_start(onesrow_b_sb[:], onesrow_b[:])
            cols_sb = consts.tile([128, 8], f32)
            nc.sync.dma_start(cols_sb[:], cols[:])
            w2_sb = consts.tile([128, 8, 128], bf16)
            nc.sync.dma_start(w2_sb[:], wq2[:].rearrange("c p k -> p c k"))
            s2_sb = consts.tile([128, 128], bf16)
            nc.sync.dma_start(s2_sb[:], s2[:])
            id_sb = consts.tile([128, 128], f32)
            nc.sync.dma_start(id_sb[:], ident[:])
            ones2_sb = consts.tile([128, 2], bf16)
            nc.sync.dma_start(ones2_sb[:], ones2b[:])

            initcol = cols_sb[:, 0:1]
            bcol = cols_sb[:, 3:4]
            iota = cols_sb[:, 4:5]
            onescol_f = cols_sb[:, 5:6]

            # ---- persistent tiles ----
            # one tile per 64-col block so each GEMM depends only on its DMA
            hts = [
                persist.tile([128, NDC, B_LOC, BT], fp8, name=f"hts{k}")
                for k in range(NB)
            ]
            # E interleaved by (round j, chain i, seq): slot (j,i) = col 31i+j
            E2I = persist.tile([128, NR + 1, NCH, B_LOC], bf16)
            emis = persist.tile([128, B_LOC, T], bf16)  # raw emisT+(b-c), rows 0:K
            OH = persist.tile([128, B_LOC, T], bf16)    # one-hot (rows 0:K)
            g1 = persist.tile([128, B_LOC], f32)        # gold emission term
            scr2 = persist.tile([128, T], bf16)         # accum scratch dst
            snapT = persist.tile([128, NCH - 1, B_LOC], bf16)
            betas = persist.tile([128, B_LOC], f32)
            wdot = persist.tile([128, B_LOC], f32)

            # ---- stage all hidden data: one DMA per 64-col block ----
            # ONE queue => per-queue in-order completion, so the first
            # blocks land early and the GEMM can stream behind the DMA.
            for k in (0, 7, 1, 6, 2, 5, 3, 4):
                nc.sync.dma_start(hts[k][:], hidtb[k])

            # ---- emissions GEMM: one unit per 64-col block (all seqs) ----
            def swap2(ap):
                """Swap the last two free-dim levels of an AP."""
                return bass.AP(ap.tensor, ap.offset,
                               ap.ap[:-2] + [ap.ap[-1], ap.ap[-2]])

            def flipd(ap, lvl):
                """Reverse free-dim level lvl (negative index) of an AP."""
                lv = list(ap.ap)
                st, n = lv[lvl]
                off = ap.offset + (n - 1) * st
                lv[lvl] = [-st, n]
                return bass.AP(ap.tensor, off, lv)

            def unit_blk(k):
                kc = slice(k * BT, (k + 1) * BT)
                pe_ = ge_ps.tile([128, B_LOC, BT], f32, tag="ge")
                for dc in range(8):
                    nc.tensor.matmul(
                        pe_[:],
                        w2_sb[:, dc, :],
                        hts[k][:, dc, :, :],
                        start=(dc == 0),
                        stop=(dc == 7),
                    )
                    if dc in (2, 5):
                        yield
                yield
                # stage the whole block into SBUF (frees the PSUM bank after
                # ONE copy); rows 0:K double as the raw gold emissions.
                nc.vector.tensor_copy(emis[:, :, kc], pe_[:])
                yield

            def unit_strip(i, bwd):
                # one Exp activation per (chain, direction) from the SBUF
                # staging: slot (j,i) = col 31i+j (fwd) / 511-31i-j (bwd).
                if not bwd:
                    a = STRIDE * i
                    nc.scalar.activation(
                        E2I[0:H2, :, i, :],
                        swap2(emis[0:H2, :, a : a + NR + 1]),
                        AF.Exp, bias=bcol[0:H2],
                    )
                else:
                    a0 = T - 1 - STRIDE * i - NR
                    nc.scalar.activation(
                        E2I[H2:128, :, i, :],
                        flipd(swap2(emis[H2:128, :, a0 : a0 + NR + 1]), -2),
                        AF.Exp, bias=bcol[H2:128],
                    )
                yield

            def unit_goldoh(k):
                # one-hot build for 64-col block k, all seqs
                kc = slice(k * BT, (k + 1) * BT)
                tagap = tag_sb[:].rearrange("p (c t) -> p c t", c=B_LOC)[:, :, kc]
                tb = ge_ps.tile([128, B_LOC, BT], f32, tag="ge")
                nc.tensor.matmul(
                    tb[0:K, :, :], onesrow_b_sb[:, 0:K], tagap,
                    start=True, stop=True,
                )
                yield
                nc.vector.tensor_scalar(
                    OH[0:K, :, kc], tb[0:K, :, :], iota[0:K], None,
                    ALU.is_equal,
                )
                yield

            def unit_goldmul(c):
                nc.gpsimd.tensor_mul(
                    OH[0:K, c, :], emis[0:K, c, :], OH[0:K, c, :]
                )
                yield
                nc.scalar.activation(
                    scr2[0:K, 0:T], OH[0:K, c, :],
                    AF.Identity, accum_out=g1[0:K, c : c + 1],
                )
                yield

            # ---- PE warm-up while the first DMA is in flight ----
            # gold one-hots first (no hidtb dependency), then junk const
            # matmuls: keeps the PE p-state ramping so the real GEMM runs
            # at full clock as soon as block 0 lands.
            for k in range(NB):
                for _ in unit_goldoh(k):
                    pass
            for _ in range(4):
                jk_ps = sc_ps.tile([128, B_LOC], f32, tag="sc", name="jk")
                nc.tensor.matmul(
                    jk_ps[:], s2_sb[:], s2_sb[:, 0:B_LOC],
                    start=True, stop=True,
                )

            # ---- pre-scan: all GEMM blocks, then E strips by availability
            for k in (0, 7, 1, 6, 2, 5, 3, 4):
                for _ in unit_blk(k):
                    pass
            for i, bwd in ((0, 0), (0, 1), (1, 0), (1, 1), (2, 1), (2, 0),
                           (3, 0), (3, 1), (4, 1), (4, 0), (5, 0), (6, 0),
                           (7, 0), (5, 1), (6, 1), (7, 1)):
                for _ in unit_strip(i, bwd):
                    pass

            # ---- chain inits: alpha[:, i, :] = E2I slot (0, i); chain 0
            # additionally multiplied by exp(start)/exp(end).
            alpha = alp_pool.tile([128, NCH, B_LOC], bf16, tag="al")
            nc.vector.tensor_copy(alpha[:, 1:, :], E2I[:, 0, 1:, :])
            nc.vector.tensor_scalar_mul(alpha[:, 0, :], E2I[:, 0, 0, :], initcol)

            # gold multiplies/accums pumped into scan gaps
            work = [unit_goldmul(c) for c in range(B_LOC)]

            def pump(n):
                for _ in range(n):
                    while work:
                        try:
                            next(work[0])
                            break
                        except StopIteration:
                            work.pop(0)

            if pump_mode == 0:
                pump(len(work) * 16)

            # ---- all chains advance with ONE matmul + ONE multiply/round --
            for j in range(1, NR + 1):
                psj = sc_ps.tile([128, NCH, B_LOC], f32, tag="sc", name="psj")
                nc.tensor.matmul(psj[:], s2_sb[:], alpha[:],
                                 start=True, stop=True)
                alpha_new = alp_pool.tile([128, NCH, B_LOC], bf16, tag="al",
                                          name="aln")
                nc.vector.tensor_mul(alpha_new[:], psj[:], E2I[:, j, :, :])
                alpha = alpha_new
                if j == WARMN:  # all handoff snapshots in one copy
                    nc.vector.tensor_copy(snapT[:], alpha[:, 1:, :])
                if pump_mode:
                    pump(pump_mode)

            pump(len(work) * 16)  # drain remaining background work

            # ---- finisher ----
            # z = alpha^_255 . (A gamma^_256) on the last chain's final state
            alf = alpha[:, NCH - 1, :]
            ps_f = sc_ps.tile([128, B_LOC], f32, tag="sc", name="psf")
            nc.tensor.matmul(ps_f[:], s2_sb[:], alf, start=True, stop=True)
            nc.vector.tensor_copy(betas[H2 : H2 + K, :], ps_f[H2 : H2 + K, :])
            psz = sc_ps.tile([128, B_LOC], f32, tag="sc", name="psz")
            nc.tensor.matmul(
                psz[0:K, :], id_sb[H2 : H2 + K, H2 : H2 + K],
                betas[H2 : H2 + K, :], start=True, stop=True,
            )
            nc.vector.tensor_mul(wdot[0:K, :], psz[0:K, :], alf[0:K, :])
            zz = sc_ps.tile([128, B_LOC], f32, tag="sc", name="zz")
            nc.tensor.matmul(zz[0:1, :], onescol_f[0:K], wdot[0:K, :],
                             start=True, stop=True)
            lnz = srow_pool.tile([1, B_LOC], f32, tag="srow")
            nc.scalar.activation(lnz[:], zz[0:1, :], AF.Ln)

            # telescoping scale ratios: sum_i [ln S(final_i) - ln S(snap_i+1)]
            # finals of chains 0..NCH-2 are alpha[:, 0:NCH-1, :].
            NH = NCH - 1
            pr_t = sc_ps.tile([128, NH * B_LOC], f32, tag="sc", name="prt")
            nc.tensor.matmul(pr_t[0:2, :], ones2_sb[:],
                             alpha[:, 0:NH, :], start=True, stop=True)
            pr_h = sc_ps.tile([128, NH * B_LOC], f32, tag="sc", name="prh")
            nc.tensor.matmul(pr_h[0:2, :], ones2_sb[:], snapT[:],
                             start=True, stop=True)
            lt_t = srow_pool.tile([2, NH, B_LOC], f32, tag="srow", name="ltt")
            nc.scalar.activation(lt_t[:], pr_t[0:2, :], AF.Ln)
            lt_h = srow_pool.tile([2, NH, B_LOC], f32, tag="srow", name="lth")
            nc.scalar.activation(lt_h[:], pr_h[0:2, :], AF.Ln)
            nc.vector.tensor_sub(lt_t[:], lt_t[:], lt_h[:])
            ps_s = sc_ps.tile([128, NH * B_LOC], f32, tag="sc", name="pss")
            nc.tensor.matmul(ps_s[0:1, :], onescol_f[0:2], lt_t[:],
                             start=True, stop=True)
            lnr = srow_pool.tile([1, NH, B_LOC], f32, tag="srow", name="lnr")
            nc.vector.tensor_copy(lnr[:], ps_s[0:1, :])

            # gold total (emission term only; host adds transition score)
            gzz = sc_ps.tile([128, B_LOC], f32, tag="sc", name="gzz")
            nc.tensor.matmul(gzz[0:1, :], onescol_f[0:K], g1[0:K, :],
                             start=True, stop=True)

            outrow = srow_pool.tile([1, B_LOC], f32, tag="srow")
            nc.vector.tensor_sub(outrow[:], lnz[:], gzz[0:1, :])
            for i in range(NCH - 1):
                nc.vector.tensor_add(outrow[:], outrow[:], lnr[:, i, :])
            nc.sync.dma_start(out_d[:], outrow[:])

    nc.compile()
    return nc


def _get_compiled(dbg=False):
    key = ("dbg" if dbg else "nc")
    if key not in _COMPILED:
        _COMPILED[key] = _build(dbg)
    return _COMPILED[key]


def _host_inputs(W, b, transitions, start_trans, end_trans):
    import ml_dtypes

    bf16 = ml_dtypes.bfloat16
    expA = np.exp(transitions).astype(np.float32)
    s2 = np.zeros((128, 128), np.float32)
    s2[0:K, 0:K] = expA
    s2[H2 : H2 + K, H2 : H2 + K] = expA.T

    wq2 = np.zeros((8, 128, 128), np.float32)
    wr = W.reshape(8, 128, K)
    wq2[:, :, 0:K] = wr
    wq2[:, :, H2 : H2 + K] = wr

    # growth-neutralising constant: E[colsum of exp(emis+b)] for h ~ N(0, I)
    c_shift = float(
        np.log(np.sum(np.exp(b.astype(np.float64)
                             + 0.5 * np.sum(W.astype(np.float64) ** 2, axis=0))))
    )

    cols = np.zeros((128, 8), np.float32)
    cols[0:K, 0] = np.exp(start_trans)
    cols[H2 : H2 + K, 0] = np.exp(end_trans)
    cols[0:K, 3] = b - c_shift
    cols[H2 : H2 + K, 3] = b - c_shift
    cols[0:K, 4] = np.arange(K, dtype=np.float32)
    cols[0:K, 5] = 1.0

    ones2 = np.zeros((128, 2), np.float32)
    ones2[0:K, 0] = 1.0
    ones2[H2 : H2 + K, 1] = 1.0

    common = {
        "wq2": np.ascontiguousarray(wq2.astype(bf16)),
        "s2": np.ascontiguousarray(s2.astype(bf16)),
        "ident": np.eye(128, dtype=np.float32),
        "cols": np.ascontiguousarray(cols),
        "ones2b": np.ascontiguousarray(ones2.astype(bf16)),
        "onesrow_b": np.ones((1, 128), bf16),
    }
    return common, c_shift


def kernel(full_hidden, tag_ids, mask, W, b, transitions, start_trans, end_trans,
           dbg=False):
    global LAST_RESULT
    import ml_dtypes
    from concourse.bass_utils import run_bass_kernel_spmd

    bf16 = ml_dtypes.bfloat16
    full_hidden = np.asarray(full_hidden, dtype=np.float32)
    tags = np.asarray(tag_ids)
    W = np.asarray(W, dtype=np.float32)
    b = np.asarray(b, dtype=np.float32)
    transitions = np.asarray(transitions, dtype=np.float32)
    start_trans = np.asarray(start_trans, dtype=np.float32)
    end_trans = np.asarray(end_trans, dtype=np.float32)

    nc = _get_compiled(dbg)
    common, c_shift = _host_inputs(W, b, transitions, start_trans, end_trans)

    # pre-transposed + blocked hidden, block-major and partition-major
    hb = full_hidden.astype(ml_dtypes.float8_e4m3fn)  # [B, T, D]
    hbt = hb.transpose(0, 2, 1).reshape(B_FULL, NDC, 128, NB, BT)
    hbt = hbt.transpose(3, 2, 1, 0, 4)               # [NB, 128, NDC, B, BT]

    in_maps = []
    for c in range(N_CORES):
        sl = slice(c * B_LOC, (c + 1) * B_LOC)
        in_maps.append(
            {
                "hidtb": np.ascontiguousarray(hbt[:, :, :, sl, :]),  # [NB,128,NDC,B,BT]
                "tagrow": np.ascontiguousarray(
                    tags[sl].astype(np.float32).reshape(1, B_LOC * T).astype(bf16)
                ),
                **common,
            }
        )

    # host-side gold transition/start/end score (depends only on tag_ids/mask)
    m = np.asarray(mask).astype(bool)
    tg = tags.astype(np.int64)
    first = tg[:, 0]
    tscore = start_trans[first].astype(np.float64)
    prev = first.copy()
    for t in range(1, T):
        step = transitions[prev, tg[:, t]]
        tscore = np.where(m[:, t], tscore + step, tscore)
        prev = np.where(m[:, t], tg[:, t], prev)
    tscore = tscore + end_trans[prev]
    # device gold accumulates RAW emissions (no bias, no -c shift), while
    # ln z carries -T*c from the E bias; reconcile both here.
    tscore = tscore + b.astype(np.float64)[tg].sum(axis=1) - T * c_shift

    res = run_bass_kernel_spmd(nc, in_maps, core_ids=list(range(N_CORES)))
    LAST_RESULT = res
    out = np.concatenate(
        [np.asarray(res.results[c]["out"]).reshape(B_LOC) for c in range(N_CORES)]
    )
    # -c_shift bias cancels between ln z (-T*c) and the gold accumulator.
    return (out.astype(np.float64) - tscore).astype(np.float32)
